# revision 5
# baseline (speedup 1.0000x reference)
"""LipschitzRNN Trainium2 kernel (v2: PSUM-resident recurrence).

Math (per reference):
    bA = 0.5*exp(-bA_z^2)+0.5 ; bW likewise
    A = (1-bA)(MA+MA.T) + bA(MA-MA.T) - YA*I
    C = (1-bA)(MW+MW.T) + bW(MW-MW.T) - YW*I
    X_{t+1} = X_t + STEP*(A@X_t + tanh(C@X_t + by))   (column-state X: [n, bs])
    out[b, t, :] = X_t[:, b]

Device strategy (8-way batch data-parallel, b=32/core, no collectives).
The baseline's serial chain was PE(C@x) -> ACT(tanh) -> DVE(xq') -> PE
(~1.3us/step).  Here the tanh input W_t = C@X_t + by is *accumulated in
PSUM* so the only per-step serial chain is ACT(tanh) -> 4 PE matmuls ->
ACT:

  W_{t+1} = W_t + STEP*(CA@x_t + C@T_t)      (T_t = tanh(W_t))

and, to keep every W-matmul except C@tq_t OFF the chain (PSUM banks are
written in place, so writers must wait for the previous tanh read -- a
WAR hazard), W lives in two parity banks updated with a 2-step formula:

  W_{t+2} = W_t + [2*STEP*CA + STEP^2*CA@A]@xq_t
                + [STEP*C + STEP^2*CA]@tq_t + STEP*C@tq_{t+1}

All terms but the last use data available ~a full step early.  X is also
PSUM-resident, accumulated by PE (STEP*A@xq + STEP*I@tq), with a single
DVE cast producing the fp16 copy xq used as matmul moving operand and as
transpose input for the output rows.  fp16 stationary rounding drift in
the W accumulator is killed by a refresh every 64 steps
(W_t := (C + STEP*CA)@xq_{t-1} + by + STEP*C@tq_{t-1}, start=True).

Output rows need [b, n] layout: one PE transpose per 2 steps
([128,128] fp16: both n-chunks x both steps of a pair), cast f16->f32
into an 8-pair staging tile, 4 DMAs per 16 steps (one per
(step-parity, n-chunk), OUT viewed as [b, 256, 2, n]).
"""

import numpy as np

N = 256
BS = 256
TMAX = 512
STEP = 0.01
YA = 0.001
YW = 0.001
NCORES = 8
BLOC = BS // NCORES  # 32
NSTEPS = TMAX - 1    # 511
REFRESH = 64

LAST_RESULT = None  # BassKernelResults of the most recent run (for test harness)


def _build():
    from concourse import bacc, tile
    import concourse.mybir as mybir

    F32 = mybir.dt.float32
    F16 = mybir.dt.float16
    AF = mybir.ActivationFunctionType

    nc = bacc.Bacc("TRN2", target_bir_lowering=False, debug=False,
                   num_devices=NCORES)

    # stationaries, all pre-transposed ([k, m] layout) and fp16
    WC1 = nc.dram_tensor("WC1", [N, N], F16, kind="ExternalInput")    # (STEP*C).T
    WCXQ = nc.dram_tensor("WCXQ", [N, N], F16, kind="ExternalInput")  # (2*STEP*CA+STEP^2*CAA).T
    WCTQ = nc.dram_tensor("WCTQ", [N, N], F16, kind="ExternalInput")  # (STEP*C+STEP^2*CA).T
    WA1 = nc.dram_tensor("WA1", [N, N], F16, kind="ExternalInput")    # (STEP*A).T
    WRF = nc.dram_tensor("WRF", [N, N], F16, kind="ExternalInput")    # (C+STEP*CA).T
    BY2 = nc.dram_tensor("BY2", [128, 128], F16, kind="ExternalInput")  # by in rows 0/1
    SEL = nc.dram_tensor("SEL", [128, 2 * BLOC], F16, kind="ExternalInput")  # chunk selector
    WI = nc.dram_tensor("WI", [128, 128], F16, kind="ExternalInput")  # diag(f16(STEP))
    IDT = nc.dram_tensor("IDT", [128, 128], F16, kind="ExternalInput")   # identity f16
    IDF = nc.dram_tensor("IDF", [128, 128], F32, kind="ExternalInput")   # identity f32
    X0SB = nc.dram_tensor("X0SB", [128, 2 * BLOC], F32, kind="ExternalInput")
    W0SB = nc.dram_tensor("W0SB", [128, 2 * BLOC], F32, kind="ExternalInput")
    W1SB = nc.dram_tensor("W1SB", [128, 2 * BLOC], F32, kind="ExternalInput")
    # OUT[b, t//2, t%2, n] == out[b, t, n]
    OUT = nc.dram_tensor("OUT", [BLOC, TMAX // 2, 2, N], F32, kind="ExternalOutput")

    B = BLOC
    NPAIRS = (NSTEPS + 1) // 2  # 256: pairs (1,2)..(509,510), pseudo-pair 255 = {511}
    NGRP = (NPAIRS + 7) // 8    # 32 groups of <=8 pairs

    with tile.TileContext(nc) as tc:
        with (
            tc.tile_pool(name="consts", bufs=1) as consts,
            tc.tile_pool(name="xqpool", bufs=2) as xqpool,
            tc.tile_pool(name="tqpool", bufs=3) as tqpool,
            tc.tile_pool(name="stpool", bufs=2) as stpool,
            tc.tile_pool(name="wev", bufs=1, space="PSUM") as wevp,
            tc.tile_pool(name="wod", bufs=1, space="PSUM") as wodp,
            tc.tile_pool(name="xps", bufs=1, space="PSUM") as xpsp,
            tc.tile_pool(name="ptp", bufs=2, space="PSUM") as ptp,
        ):
            # ---- constants ----
            def load4(dram, tag):
                ts = [[consts.tile([128, 128], F16, tag=f"{tag}{k}{m}", name=f"{tag}{k}{m}")
                       for m in range(2)] for k in range(2)]
                for k in range(2):
                    for m in range(2):
                        nc.sync.dma_start(
                            ts[k][m][:],
                            dram[128 * k:128 * (k + 1), 128 * m:128 * (m + 1)])
                return ts

            wc1 = load4(WC1, "wc1")
            wcxq = load4(WCXQ, "wcxq")
            wctq = load4(WCTQ, "wctq")
            wa1 = load4(WA1, "wa1")
            wrf = load4(WRF, "wrf")
            by2 = consts.tile([128, 128], F16, tag="by2")
            nc.sync.dma_start(by2[:], BY2[:, :])
            sel = consts.tile([128, 2 * B], F16, tag="sel")
            nc.sync.dma_start(sel[:], SEL[:, :])
            wi = consts.tile([128, 128], F16, tag="wi")
            nc.sync.dma_start(wi[:], WI[:, :])
            idt = consts.tile([128, 128], F16, tag="idt")
            nc.sync.dma_start(idt[:], IDT[:, :])
            idf = consts.tile([128, 128], F32, tag="idf")
            nc.sync.dma_start(idf[:], IDF[:, :])
            x0sb = consts.tile([128, 2 * B], F32, tag="x0sb")
            nc.sync.dma_start(x0sb[:], X0SB[:, :])
            w0sb = consts.tile([128, 2 * B], F32, tag="w0sb")
            nc.sync.dma_start(w0sb[:], W0SB[:, :])
            w1sb = consts.tile([128, 2 * B], F32, tag="w1sb")
            nc.sync.dma_start(w1sb[:], W1SB[:, :])

            # ---- persistent PSUM state ----
            wev = wevp.tile([128, 2 * B], F32, tag="wev")   # W_t, even t
            wod = wodp.tile([128, 2 * B], F32, tag="wod")   # W_t, odd t
            xp = xpsp.tile([128, 2 * B], F32, tag="xp")     # X_t

            # init: W_0 (even), W_1 (odd, host-computed), X_0 via identity mm
            nc.tensor.matmul(wev[:], idf[:], w0sb[:], start=True, stop=True)
            nc.tensor.matmul(wod[:], idf[:], w1sb[:], start=True, stop=True)
            nc.tensor.matmul(xp[:], idf[:], x0sb[:], start=True, stop=True)

            # xq_0 / tq_0
            xq0 = consts.tile([128, 2 * B], F16, tag="xq0")
            nc.vector.tensor_copy(xq0[:], x0sb[:])
            tq = {}
            tq[0] = tqpool.tile([128, 2 * B], F16, tag="tq", name="tq")
            nc.scalar.activation(tq[0][:, 0:B], wev[:, 0:B], AF.Tanh,
                                 bias=0.0, scale=1.0)
            nc.scalar.activation(tq[0][:, B:2 * B], wev[:, B:2 * B], AF.Tanh,
                                 bias=0.0, scale=1.0)

            pair_tiles = {}
            pt_tiles = {}
            stage_tiles = {}

            def xq_ref(t):
                if t == 0:
                    return xq0[:, 0:2 * B]
                p, sp = (t - 1) // 2, (t - 1) % 2
                return pair_tiles[p][:, sp * 2 * B:(sp + 1) * 2 * B]

            def wbank(t):
                return wev if t % 2 == 0 else wod

            def acc_mm(out_region, lhsT, rhs, start=False):
                nc.tensor.matmul(out_region, lhsT, rhs, start=start,
                                 stop=True, skip_group_check=True)

            # emission schedules for the output path
            tp_emit = {}
            cast_emit = {}
            dma_emit = {}
            for p in range(NPAIRS - 1):           # real pairs 0..254
                tp_emit.setdefault(2 * p + 3, []).append(p)
                te = 2 * p + 4
                cast_emit.setdefault(te if te <= NSTEPS else -1, []).append(p)
            for g in range(NGRP - 1):             # groups 0..30
                dma_emit.setdefault(16 * g + 18, []).append(g)

            def emit_tp(p, cols):
                pt = ptp.tile([128, 128], F16, tag="pt", name="pt")
                pt_tiles[p] = pt
                nc.tensor.transpose(pt[0:cols, :], pair_tiles[p][:, 0:cols],
                                    idt[:])

            def emit_cast(p, parts):
                g, s = p // 8, p % 8
                if g not in stage_tiles:
                    stage_tiles[g] = stpool.tile([128, 8, 128], F32, tag="stage", name="stage")
                nc.vector.tensor_copy(stage_tiles[g][0:parts, s, :],
                                      pt_tiles[p][0:parts, :])
                del pt_tiles[p]

            def emit_dma(g, npair0, npair1):
                st = stage_tiles[g]
                for c in range(2):
                    # odd steps t=2p+1 -> OUT[:, p, 1, :]
                    nc.sync.dma_start(
                        OUT[:, 8 * g:8 * g + npair0, 1, 128 * c:128 * (c + 1)],
                        st[64 * 0 + 32 * c:64 * 0 + 32 * c + 32, 0:npair0, :])
                    # even steps t=2p+2 -> OUT[:, p+1, 0, :]
                    nc.sync.dma_start(
                        OUT[:, 8 * g + 1:8 * g + 1 + npair1, 0, 128 * c:128 * (c + 1)],
                        st[64 * 1 + 32 * c:64 * 1 + 32 * c + 32, 0:npair1, :])

            # ---- main loop: iter t produces W_t (t>=2), X_t, tanh_t, xq_t ----
            for t in range(1, NSTEPS + 1):
                is_rf = (t % REFRESH) in (0, 1)
                nx_rf = ((t + 1) % REFRESH) in (0, 1)
                wb = wbank(t)
                tqp = tq[t - 1]
                xqp = xq_ref(t - 1)

                # chain matmuls completing W_t (and refresh prologue)
                if 2 <= t <= NSTEPS - 1:
                    if is_rf:
                        # W := by (one start=True mm covering the full
                        # [*,0:64] footprint -- zero regions are coarser
                        # than one m-chunk), then accumulate C+STEP*CA
                        acc_mm(wb[:, 0:2 * B], by2[:], sel[:], start=True)
                        for m in range(2):
                            r = wb[:, m * B:(m + 1) * B]
                            acc_mm(r, wrf[0][m][:], xqp[:, 0:B])
                            acc_mm(r, wrf[1][m][:], xqp[:, B:2 * B])
                    for k in range(2):
                        for m in range(2):
                            acc_mm(wb[:, m * B:(m + 1) * B], wc1[k][m][:],
                                   tqp[:, k * B:(k + 1) * B])

                # X_t accumulation
                for m in range(2):
                    r = xp[:, m * B:(m + 1) * B]
                    acc_mm(r, wi[:], tqp[:, m * B:(m + 1) * B])
                    acc_mm(r, wa1[0][m][:], xqp[:, 0:B])
                    acc_mm(r, wa1[1][m][:], xqp[:, B:2 * B])

                # off-chain 2-step terms for W_{t+1}
                if t + 1 <= NSTEPS - 1 and not nx_rf:
                    wn = wbank(t + 1)
                    for m in range(2):
                        r = wn[:, m * B:(m + 1) * B]
                        acc_mm(r, wcxq[0][m][:], xqp[:, 0:B])
                        acc_mm(r, wcxq[1][m][:], xqp[:, B:2 * B])
                        acc_mm(r, wctq[0][m][:], tqp[:, 0:B])
                        acc_mm(r, wctq[1][m][:], tqp[:, B:2 * B])

                # tanh_t
                if t <= NSTEPS - 1:
                    tq[t] = tqpool.tile([128, 2 * B], F16, tag="tq", name="tq")
                    nc.scalar.activation(tq[t][:, 0:B], wb[:, 0:B], AF.Tanh,
                                         bias=0.0, scale=1.0)
                    nc.scalar.activation(tq[t][:, B:2 * B], wb[:, B:2 * B],
                                         AF.Tanh, bias=0.0, scale=1.0)
                tq.pop(t - 2, None)

                # xq_t (DVE, off-chain)
                p, sp = (t - 1) // 2, (t - 1) % 2
                if sp == 0:
                    pair_tiles[p] = xqpool.tile([128, 4 * B], F16, tag="xqpair", name="xqpair")
                nc.vector.tensor_copy(
                    pair_tiles[p][:, sp * 2 * B:(sp + 1) * 2 * B], xp[:])

                # output path work scheduled for this iter
                for pp in tp_emit.get(t, ()):
                    emit_tp(pp, 128)
                for pp in cast_emit.get(t, ()):
                    emit_cast(pp, 128)
                for g in dma_emit.get(t, ()):
                    emit_dma(g, 8, 8)
                    del stage_tiles[g]

            # ---- tail: pseudo-pair 255 (step 511), leftover casts, group 31
            emit_tp(NPAIRS - 1, 64)
            for pp in cast_emit.get(-1, ()):
                emit_cast(pp, 128)
            emit_cast(NPAIRS - 1, 64)
            emit_dma(NGRP - 1, 8, 7)

    nc.compile()
    return nc


def kernel(X0, MA, MW, bA_z, bW_z, by_w):
    global LAST_RESULT
    from concourse.bass_utils import run_bass_kernel_spmd

    X0 = np.asarray(X0, dtype=np.float32)
    MA = np.asarray(MA, dtype=np.float32)
    MW = np.asarray(MW, dtype=np.float32)
    bA_z = np.asarray(bA_z, dtype=np.float32)
    bW_z = np.asarray(bW_z, dtype=np.float32)
    by = np.asarray(by_w, dtype=np.float32)

    bA = np.float32(0.5) * np.exp(-bA_z[0, 0] * bA_z[0, 0]) + np.float32(0.5)
    bW = np.float32(0.5) * np.exp(-bW_z[0, 0] * bW_z[0, 0]) + np.float32(0.5)
    I = np.eye(N, dtype=np.float32)
    A = (1 - bA) * (MA + MA.T) + bA * (MA - MA.T) - np.float32(YA) * I
    C = (1 - bA) * (MW + MW.T) + bW * (MW - MW.T) - np.float32(YW) * I

    A64, C64 = A.astype(np.float64), C.astype(np.float64)
    CA = C64 @ A64
    CAA = CA @ A64

    def f16T(M):
        return np.ascontiguousarray(M.T.astype(np.float32)).astype(np.float16)

    WC1 = f16T(STEP * C64)
    WCXQ = f16T(2 * STEP * CA + STEP * STEP * CAA)
    WCTQ = f16T(STEP * C64 + STEP * STEP * CA)
    WA1 = f16T(STEP * A64)
    WRF = f16T(C64 + STEP * CA)
    BY2 = np.zeros((128, 128), dtype=np.float16)
    BY2[0, :] = by[0:128, 0].astype(np.float16)
    BY2[1, :] = by[128:256, 0].astype(np.float16)
    SEL = np.zeros((128, 2 * BLOC), dtype=np.float16)
    SEL[0, 0:BLOC] = 1
    SEL[1, BLOC:2 * BLOC] = 1
    WI = (np.eye(128) * np.float16(STEP)).astype(np.float16)
    IDT = np.eye(128, dtype=np.float16)
    IDF = np.eye(128, dtype=np.float32)
    ONES = np.ones((128, BLOC), dtype=np.float16)

    CA32 = CA.astype(np.float32)

    def pack(M):  # [256, b] -> [128, 2b] chunk-major
        return np.concatenate([M[0:128, :], M[128:256, :]], axis=1)

    in_maps = []
    for i in range(NCORES):
        X0c = np.ascontiguousarray(X0[i * BLOC:(i + 1) * BLOC, :].T)  # [256, 32]
        W0 = C @ X0c + by
        xq0 = X0c.astype(np.float16).astype(np.float32)
        tq0 = np.tanh(W0).astype(np.float16).astype(np.float32)
        W1 = W0 + np.float32(STEP) * (CA32 @ xq0 + C @ tq0)
        in_maps.append({
            "WC1": WC1, "WCXQ": WCXQ, "WCTQ": WCTQ, "WA1": WA1, "WRF": WRF,
            "BY2": BY2, "SEL": SEL, "WI": WI, "IDT": IDT, "IDF": IDF,
            "X0SB": np.ascontiguousarray(pack(X0c)),
            "W0SB": np.ascontiguousarray(pack(W0.astype(np.float32))),
            "W1SB": np.ascontiguousarray(pack(W1.astype(np.float32))),
        })

    nc = _build()
    res = run_bass_kernel_spmd(nc, in_maps, core_ids=list(range(NCORES)))
    LAST_RESULT = res

    out = np.concatenate(
        [r["OUT"].reshape(BLOC, TMAX, N) for r in res.results], axis=0)
    out[:, 0, :] = X0
    return out


if __name__ == "__main__":
    rng = np.random.default_rng(0)
    inputs = {
        "X0": rng.standard_normal((BS, N), dtype=np.float32),
        "MA": rng.standard_normal((N, N), dtype=np.float32) / 16,
        "MW": rng.standard_normal((N, N), dtype=np.float32) / 16,
        "bA_z": np.full((1, 1), 0.65, dtype=np.float32),
        "bW_z": np.full((1, 1), 0.65, dtype=np.float32),
        "by_w": rng.standard_normal((N, 1), dtype=np.float32) / 100,
    }
    out = kernel(**inputs)
    print("out", out.shape, out.dtype, np.abs(out).max())


# revision 6
# speedup vs baseline: 1.2097x; 1.2097x over previous
"""LipschitzRNN Trainium2 kernel (v2: PSUM-resident recurrence).

Math (per reference):
    bA = 0.5*exp(-bA_z^2)+0.5 ; bW likewise
    A = (1-bA)(MA+MA.T) + bA(MA-MA.T) - YA*I
    C = (1-bA)(MW+MW.T) + bW(MW-MW.T) - YW*I
    X_{t+1} = X_t + STEP*(A@X_t + tanh(C@X_t + by))   (column-state X: [n, bs])
    out[b, t, :] = X_t[:, b]

Device strategy (8-way batch data-parallel, b=32/core, no collectives).
The baseline's serial chain was PE(C@x) -> ACT(tanh) -> DVE(xq') -> PE
(~1.3us/step).  Here the tanh input W_t = C@X_t + by is *accumulated in
PSUM* so the only per-step serial chain is ACT(tanh) -> 4 PE matmuls ->
ACT:

  W_{t+1} = W_t + STEP*(CA@x_t + C@T_t)      (T_t = tanh(W_t))

and, to keep every W-matmul except C@tq_t OFF the chain (PSUM banks are
written in place, so writers must wait for the previous tanh read -- a
WAR hazard), W lives in two parity banks updated with a 2-step formula:

  W_{t+2} = W_t + [2*STEP*CA + STEP^2*CA@A]@xq_t
                + [STEP*C + STEP^2*CA]@tq_t + STEP*C@tq_{t+1}

All terms but the last use data available ~a full step early.  X is also
PSUM-resident, accumulated by PE (STEP*A@xq + STEP*I@tq), with a single
DVE cast producing the fp16 copy xq used as matmul moving operand and as
transpose input for the output rows.  fp16 stationary rounding drift in
the W accumulator is killed by a refresh every 64 steps
(W_t := (C + STEP*CA)@xq_{t-1} + by + STEP*C@tq_{t-1}, start=True).

Output rows need [b, n] layout: one PE transpose per 2 steps
([128,128] fp16: both n-chunks x both steps of a pair), cast f16->f32
into an 8-pair staging tile, 4 DMAs per 16 steps (one per
(step-parity, n-chunk), OUT viewed as [b, 256, 2, n]).
"""

import numpy as np

N = 256
BS = 256
TMAX = 512
STEP = 0.01
YA = 0.001
YW = 0.001
NCORES = 8
BLOC = BS // NCORES  # 32
NSTEPS = TMAX - 1    # 511
REFRESH = 64

LAST_RESULT = None  # BassKernelResults of the most recent run (for test harness)


def _build():
    from concourse import bacc, tile
    import concourse.mybir as mybir

    F32 = mybir.dt.float32
    F16 = mybir.dt.float16
    BF16 = mybir.dt.bfloat16
    AF = mybir.ActivationFunctionType

    nc = bacc.Bacc("TRN2", target_bir_lowering=False, debug=False,
                   num_devices=NCORES)

    # stationaries, all pre-transposed ([k, m] layout) and fp16
    WC1 = nc.dram_tensor("WC1", [N, N], BF16, kind="ExternalInput")    # (STEP*C).T
    WCXQ = nc.dram_tensor("WCXQ", [N, N], F16, kind="ExternalInput")  # (2*STEP*CA+STEP^2*CAA).T
    WCTQ = nc.dram_tensor("WCTQ", [N, N], BF16, kind="ExternalInput")  # (STEP*C+STEP^2*CA).T
    WA1 = nc.dram_tensor("WA1", [N, N], F16, kind="ExternalInput")    # (STEP*A).T
    WRF = nc.dram_tensor("WRF", [N, N], F16, kind="ExternalInput")    # (C+STEP*CA).T
    BY2 = nc.dram_tensor("BY2", [128, 128], F16, kind="ExternalInput")  # by in rows 0/1
    SEL = nc.dram_tensor("SEL", [128, 2 * BLOC], F16, kind="ExternalInput")  # chunk selector
    WI = nc.dram_tensor("WI", [128, 128], BF16, kind="ExternalInput")  # diag(f16(STEP))
    IDT = nc.dram_tensor("IDT", [128, 128], F16, kind="ExternalInput")   # identity f16
    IDF = nc.dram_tensor("IDF", [128, 128], F32, kind="ExternalInput")   # identity f32
    X0SB = nc.dram_tensor("X0SB", [128, 2 * BLOC], F32, kind="ExternalInput")
    W0SB = nc.dram_tensor("W0SB", [128, 2 * BLOC], F32, kind="ExternalInput")
    W1SB = nc.dram_tensor("W1SB", [128, 2 * BLOC], F32, kind="ExternalInput")
    # OUT[b, t//2, t%2, n] == out[b, t, n]
    OUT = nc.dram_tensor("OUT", [BLOC, TMAX // 2, 2, N], F32, kind="ExternalOutput")

    B = BLOC
    NPAIRS = (NSTEPS + 1) // 2  # 256: pairs (1,2)..(509,510), pseudo-pair 255 = {511}
    NGRP = (NPAIRS + 7) // 8    # 32 groups of <=8 pairs

    with tile.TileContext(nc) as tc:
        with (
            tc.tile_pool(name="consts", bufs=1) as consts,
            tc.tile_pool(name="xqpool", bufs=2) as xqpool,
            tc.tile_pool(name="tqpool", bufs=3) as tqpool,
            tc.tile_pool(name="stpool", bufs=2) as stpool,
            tc.tile_pool(name="wev", bufs=1, space="PSUM") as wevp,
            tc.tile_pool(name="wod", bufs=1, space="PSUM") as wodp,
            tc.tile_pool(name="xps", bufs=1, space="PSUM") as xpsp,
            tc.tile_pool(name="ptp", bufs=2, space="PSUM") as ptp,
        ):
            # ---- constants ----
            def load4(dram, tag, dt=F16):
                ts = [[consts.tile([128, 128], dt, tag=f"{tag}{k}{m}", name=f"{tag}{k}{m}")
                       for m in range(2)] for k in range(2)]
                for k in range(2):
                    for m in range(2):
                        nc.sync.dma_start(
                            ts[k][m][:],
                            dram[128 * k:128 * (k + 1), 128 * m:128 * (m + 1)])
                return ts

            wc1 = load4(WC1, "wc1", BF16)
            wcxq = load4(WCXQ, "wcxq")
            wctq = load4(WCTQ, "wctq", BF16)
            wa1 = load4(WA1, "wa1")
            wrf = load4(WRF, "wrf")
            by2 = consts.tile([128, 128], F16, tag="by2")
            nc.sync.dma_start(by2[:], BY2[:, :])
            sel = consts.tile([128, 2 * B], F16, tag="sel")
            nc.sync.dma_start(sel[:], SEL[:, :])
            wi = consts.tile([128, 128], BF16, tag="wi")
            nc.sync.dma_start(wi[:], WI[:, :])
            idt = consts.tile([128, 128], F16, tag="idt")
            nc.sync.dma_start(idt[:], IDT[:, :])
            idf = consts.tile([128, 128], F32, tag="idf")
            nc.sync.dma_start(idf[:], IDF[:, :])
            x0sb = consts.tile([128, 2 * B], F32, tag="x0sb")
            nc.sync.dma_start(x0sb[:], X0SB[:, :])
            w0sb = consts.tile([128, 2 * B], F32, tag="w0sb")
            nc.sync.dma_start(w0sb[:], W0SB[:, :])
            w1sb = consts.tile([128, 2 * B], F32, tag="w1sb")
            nc.sync.dma_start(w1sb[:], W1SB[:, :])

            # ---- persistent PSUM state ----
            wev = wevp.tile([128, 2 * B], F32, tag="wev")   # W_t, even t
            wod = wodp.tile([128, 2 * B], F32, tag="wod")   # W_t, odd t
            xp = xpsp.tile([128, 2 * B], F32, tag="xp")     # X_t

            # init: W_0 (even), W_1 (odd, host-computed), X_0 via identity mm
            nc.tensor.matmul(wev[:], idf[:], w0sb[:], start=True, stop=True)
            nc.tensor.matmul(wod[:], idf[:], w1sb[:], start=True, stop=True)
            nc.tensor.matmul(xp[:], idf[:], x0sb[:], start=True, stop=True)

            # xq_0 / tq_0
            xq0 = consts.tile([128, 2 * B], F16, tag="xq0")
            nc.vector.tensor_copy(xq0[:], x0sb[:])
            tq = {}
            tq[0] = tqpool.tile([128, 2 * B], BF16, tag="tq", name="tq")
            nc.scalar.activation(tq[0][:], wev[:], AF.Tanh, bias=0.0, scale=1.0)

            pair_tiles = {}
            pt_tiles = {}
            stage_tiles = {}

            def xq_ref(t):
                if t == 0:
                    return xq0[:, 0:2 * B]
                p, sp = (t - 1) // 2, (t - 1) % 2
                return pair_tiles[p][:, sp * 2 * B:(sp + 1) * 2 * B]

            def wbank(t):
                return wev if t % 2 == 0 else wod

            def acc_mm(out_region, lhsT, rhs, start=False):
                nc.tensor.matmul(out_region, lhsT, rhs, start=start,
                                 stop=True, skip_group_check=True)

            # emission schedules for the output path
            tp_emit = {}
            cast_emit = {}
            dma_emit = {}
            for p in range(NPAIRS - 1):           # real pairs 0..254
                tp_emit.setdefault(2 * p + 3, []).append(p)
                te = 2 * p + 4
                cast_emit.setdefault(te if te <= NSTEPS else -1, []).append(p)
            for g in range(NGRP - 1):             # groups 0..30
                dma_emit.setdefault(16 * g + 18, []).append(g)

            def emit_tp(p, cols):
                pt = ptp.tile([128, 128], F16, tag="pt", name="pt")
                pt_tiles[p] = pt
                nc.tensor.transpose(pt[0:cols, :], pair_tiles[p][:, 0:cols],
                                    idt[:])

            def emit_cast(p, parts):
                g, s = p // 8, p % 8
                if g not in stage_tiles:
                    stage_tiles[g] = stpool.tile([128, 8, 128], F32, tag="stage", name="stage")
                nc.vector.tensor_copy(stage_tiles[g][0:parts, s, :],
                                      pt_tiles[p][0:parts, :])
                del pt_tiles[p]

            def emit_dma(g, npair0, npair1):
                st = stage_tiles[g]
                for c in range(2):
                    # odd steps t=2p+1 -> OUT[:, p, 1, :]
                    nc.sync.dma_start(
                        OUT[:, 8 * g:8 * g + npair0, 1, 128 * c:128 * (c + 1)],
                        st[64 * 0 + 32 * c:64 * 0 + 32 * c + 32, 0:npair0, :])
                    # even steps t=2p+2 -> OUT[:, p+1, 0, :]
                    nc.sync.dma_start(
                        OUT[:, 8 * g + 1:8 * g + 1 + npair1, 0, 128 * c:128 * (c + 1)],
                        st[64 * 1 + 32 * c:64 * 1 + 32 * c + 32, 0:npair1, :])

            # ---- main loop: iter t produces W_t (t>=2), X_t, tanh_t, xq_t ----
            for t in range(1, NSTEPS + 1):
                is_rf = (t % REFRESH) in (0, 1)
                nx_rf = ((t + 1) % REFRESH) in (0, 1)
                wb = wbank(t)
                tqp = tq[t - 1]
                xqp = xq_ref(t - 1)

                # chain matmuls completing W_t (and refresh prologue)
                if 2 <= t <= NSTEPS - 1:
                    if is_rf:
                        # W := by (one start=True mm covering the full
                        # [*,0:64] footprint -- zero regions are coarser
                        # than one m-chunk), then accumulate C+STEP*CA
                        acc_mm(wb[:, 0:2 * B], by2[:], sel[:], start=True)
                        for m in range(2):
                            r = wb[:, m * B:(m + 1) * B]
                            acc_mm(r, wrf[0][m][:], xqp[:, 0:B])
                            acc_mm(r, wrf[1][m][:], xqp[:, B:2 * B])
                    for k in range(2):
                        for m in range(2):
                            acc_mm(wb[:, m * B:(m + 1) * B], wc1[k][m][:],
                                   tqp[:, k * B:(k + 1) * B])

                # X_t accumulation
                for m in range(2):
                    r = xp[:, m * B:(m + 1) * B]
                    acc_mm(r, wi[:], tqp[:, m * B:(m + 1) * B])
                    acc_mm(r, wa1[0][m][:], xqp[:, 0:B])
                    acc_mm(r, wa1[1][m][:], xqp[:, B:2 * B])

                # off-chain 2-step terms for W_{t+1}
                if t + 1 <= NSTEPS - 1 and not nx_rf:
                    wn = wbank(t + 1)
                    for m in range(2):
                        r = wn[:, m * B:(m + 1) * B]
                        acc_mm(r, wcxq[0][m][:], xqp[:, 0:B])
                        acc_mm(r, wcxq[1][m][:], xqp[:, B:2 * B])
                        acc_mm(r, wctq[0][m][:], tqp[:, 0:B])
                        acc_mm(r, wctq[1][m][:], tqp[:, B:2 * B])

                # tanh_t
                if t <= NSTEPS - 1:
                    tq[t] = tqpool.tile([128, 2 * B], BF16, tag="tq", name="tq")
                    nc.scalar.activation(tq[t][:], wb[:], AF.Tanh,
                                         bias=0.0, scale=1.0)
                tq.pop(t - 2, None)

                # xq_t (DVE, off-chain)
                p, sp = (t - 1) // 2, (t - 1) % 2
                if sp == 0:
                    pair_tiles[p] = xqpool.tile([128, 4 * B], F16, tag="xqpair", name="xqpair")
                nc.vector.tensor_copy(
                    pair_tiles[p][:, sp * 2 * B:(sp + 1) * 2 * B], xp[:])

                # output path work scheduled for this iter
                for pp in tp_emit.get(t, ()):
                    emit_tp(pp, 128)
                for pp in cast_emit.get(t, ()):
                    emit_cast(pp, 128)
                for g in dma_emit.get(t, ()):
                    emit_dma(g, 8, 8)
                    del stage_tiles[g]

            # ---- tail: pseudo-pair 255 (step 511), leftover casts, group 31
            emit_tp(NPAIRS - 1, 64)
            for pp in cast_emit.get(-1, ()):
                emit_cast(pp, 128)
            emit_cast(NPAIRS - 1, 64)
            emit_dma(NGRP - 1, 8, 7)

    nc.compile()
    return nc


def kernel(X0, MA, MW, bA_z, bW_z, by_w):
    global LAST_RESULT
    from concourse.bass_utils import run_bass_kernel_spmd

    X0 = np.asarray(X0, dtype=np.float32)
    MA = np.asarray(MA, dtype=np.float32)
    MW = np.asarray(MW, dtype=np.float32)
    bA_z = np.asarray(bA_z, dtype=np.float32)
    bW_z = np.asarray(bW_z, dtype=np.float32)
    by = np.asarray(by_w, dtype=np.float32)

    bA = np.float32(0.5) * np.exp(-bA_z[0, 0] * bA_z[0, 0]) + np.float32(0.5)
    bW = np.float32(0.5) * np.exp(-bW_z[0, 0] * bW_z[0, 0]) + np.float32(0.5)
    I = np.eye(N, dtype=np.float32)
    A = (1 - bA) * (MA + MA.T) + bA * (MA - MA.T) - np.float32(YA) * I
    C = (1 - bA) * (MW + MW.T) + bW * (MW - MW.T) - np.float32(YW) * I

    A64, C64 = A.astype(np.float64), C.astype(np.float64)
    CA = C64 @ A64
    CAA = CA @ A64

    import ml_dtypes

    def f16T(M):
        return np.ascontiguousarray(M.T.astype(np.float32)).astype(np.float16)

    def bf16T(M):
        return np.ascontiguousarray(M.T.astype(np.float32)).astype(ml_dtypes.bfloat16)

    WC1 = bf16T(STEP * C64)
    WCXQ = f16T(2 * STEP * CA + STEP * STEP * CAA)
    WCTQ = bf16T(STEP * C64 + STEP * STEP * CA)
    WA1 = f16T(STEP * A64)
    WRF = f16T(C64 + STEP * CA)
    BY2 = np.zeros((128, 128), dtype=np.float16)
    BY2[0, :] = by[0:128, 0].astype(np.float16)
    BY2[1, :] = by[128:256, 0].astype(np.float16)
    SEL = np.zeros((128, 2 * BLOC), dtype=np.float16)
    SEL[0, 0:BLOC] = 1
    SEL[1, BLOC:2 * BLOC] = 1
    WI = (np.eye(128, dtype=np.float32) * np.float32(STEP)).astype(ml_dtypes.bfloat16)
    IDT = np.eye(128, dtype=np.float16)
    IDF = np.eye(128, dtype=np.float32)
    ONES = np.ones((128, BLOC), dtype=np.float16)

    CA32 = CA.astype(np.float32)

    def pack(M):  # [256, b] -> [128, 2b] chunk-major
        return np.concatenate([M[0:128, :], M[128:256, :]], axis=1)

    in_maps = []
    for i in range(NCORES):
        X0c = np.ascontiguousarray(X0[i * BLOC:(i + 1) * BLOC, :].T)  # [256, 32]
        W0 = C @ X0c + by
        xq0 = X0c.astype(np.float16).astype(np.float32)
        tq0 = np.tanh(W0).astype(np.float16).astype(np.float32)
        W1 = W0 + np.float32(STEP) * (CA32 @ xq0 + C @ tq0)
        in_maps.append({
            "WC1": WC1, "WCXQ": WCXQ, "WCTQ": WCTQ, "WA1": WA1, "WRF": WRF,
            "BY2": BY2, "SEL": SEL, "WI": WI, "IDT": IDT, "IDF": IDF,
            "X0SB": np.ascontiguousarray(pack(X0c)),
            "W0SB": np.ascontiguousarray(pack(W0.astype(np.float32))),
            "W1SB": np.ascontiguousarray(pack(W1.astype(np.float32))),
        })

    nc = _build()
    res = run_bass_kernel_spmd(nc, in_maps, core_ids=list(range(NCORES)))
    LAST_RESULT = res

    out = np.concatenate(
        [r["OUT"].reshape(BLOC, TMAX, N) for r in res.results], axis=0)
    out[:, 0, :] = X0
    return out


if __name__ == "__main__":
    rng = np.random.default_rng(0)
    inputs = {
        "X0": rng.standard_normal((BS, N), dtype=np.float32),
        "MA": rng.standard_normal((N, N), dtype=np.float32) / 16,
        "MW": rng.standard_normal((N, N), dtype=np.float32) / 16,
        "bA_z": np.full((1, 1), 0.65, dtype=np.float32),
        "bW_z": np.full((1, 1), 0.65, dtype=np.float32),
        "by_w": rng.standard_normal((N, 1), dtype=np.float32) / 100,
    }
    out = kernel(**inputs)
    print("out", out.shape, out.dtype, np.abs(out).max())
